# revision 1
# baseline (speedup 1.0000x reference)
"""Trainium2 Bass kernel for Physics-Attention over an irregular mesh.

Contract: kernel(**inputs) takes the FULL inputs from setup_inputs() and
returns the FULL [4, 32768, 256] f32 output, distributing across 8 cores
internally (one (batch, half-of-N) shard per core, pairwise AllReduce on the
slice-token pooling reductions).
"""

import sys

sys.path.insert(0, "/opt/trn_rl_repo")

import numpy as np
import ml_dtypes

import concourse.bass as bass
import concourse.mybir as mybir
import concourse.tile as tile
from concourse import bacc, bass_utils
from concourse.bass import ts

F32 = mybir.dt.float32
BF16 = mybir.dt.bfloat16
AF = mybir.ActivationFunctionType
ALU = mybir.AluOpType

B, N, DIM = 4, 32768, 256
H, D, G = 8, 64, 64
INNER = H * D  # 512
NCORES = 8
NLOC = N // 2          # 16384 tokens per core
TOK = 128              # tokens per tile
T = NLOC // TOK        # 128 tiles
KCH = DIM // 128       # 2 contraction chunks
EPS_SLICE = 1e-5

_CACHE = {}


def _build(attn_scale: float, res_scale: float, debug: bool = False):
    """Build the single-core SPMD program (identical on all 8 cores)."""
    nc = bacc.Bacc("TRN2", target_bir_lowering=False, debug=False,
                   enable_asserts=False, num_devices=NCORES)

    xT_d = nc.dram_tensor("xT", [DIM, NLOC], BF16, kind="ExternalInput").ap()
    AT_d = nc.dram_tensor("AT", [DIM, INNER], BF16, kind="ExternalInput").ap()
    WfxT_d = nc.dram_tensor("WfxT", [DIM, INNER], BF16, kind="ExternalInput").ap()
    idbf_d = nc.dram_tensor("idbf", [128, 128], BF16, kind="ExternalInput").ap()
    id32_d = nc.dram_tensor("id32", [64, 64], F32, kind="ExternalInput").ap()
    WqT_d = nc.dram_tensor("WqT", [D, D], F32, kind="ExternalInput").ap()
    WkT_d = nc.dram_tensor("WkT", [D, D], F32, kind="ExternalInput").ap()
    WvT_d = nc.dram_tensor("WvT", [D, D], F32, kind="ExternalInput").ap()
    WoT_d = nc.dram_tensor("WoT", [INNER, DIM], BF16, kind="ExternalInput").ap()
    out_d = nc.dram_tensor("out", [NLOC, DIM], F32, kind="ExternalOutput").ap()
    if debug:
        dbg_swn_d = nc.dram_tensor("dbg_swn", [128, INNER], BF16, kind="ExternalOutput").ap()
        dbg_usw_d = nc.dram_tensor("dbg_usw", [128, INNER], BF16, kind="ExternalOutput").ap()
        dbg_stg_d = nc.dram_tensor("dbg_stg", [64, H * (D + 1)], F32, kind="ExternalOutput").ap()
        dbg_C_d = nc.dram_tensor("dbg_C", [128, 4 * DIM], BF16, kind="ExternalOutput").ap()
        dbg_ost_d = nc.dram_tensor("dbg_ost", [64, H * D], F32, kind="ExternalOutput").ap()

    xT_v = xT_d.rearrange("(c p) n -> p c n", p=128)    # [128, 2, NLOC]
    AT_v = AT_d.rearrange("(c p) n -> p c n", p=128)    # [128, 2, 512]
    WfxT_v = WfxT_d.rearrange("(c p) n -> p c n", p=128)
    WoT_v = WoT_d.rearrange("(h d) f -> d h f", d=64)   # [64, 8, 256]
    out_v = out_d.rearrange("(t p) f -> t p f", p=TOK)  # [T, 128, 256]

    with tile.TileContext(nc) as tc:
        with (
            tc.tile_pool(name="consts", bufs=1) as consts,
            tc.tile_pool(name="store", bufs=1) as store,
            tc.tile_pool(name="work", bufs=3) as work,
            tc.tile_pool(name="small", bufs=3) as small,
            tc.tile_pool(name="stage", bufs=1) as stg_pool,
            tc.tile_pool(name="psmm", bufs=2, space="PSUM") as psmm,
            tc.tile_pool(name="psacc", bufs=1, space="PSUM") as psacc,
            tc.tile_pool(name="dram", bufs=1, space="DRAM") as dram,
        ):
            # ---- resident constants ----
            AT_sb = consts.tile([128, KCH, INNER], BF16)
            nc.sync.dma_start(AT_sb, AT_v)
            WfxT_sb = consts.tile([128, KCH, INNER], BF16)
            nc.sync.dma_start(WfxT_sb, WfxT_v)
            idbf = consts.tile([128, 128], BF16)
            nc.sync.dma_start(idbf, idbf_d)
            id32 = consts.tile([64, 64], F32)
            nc.sync.dma_start(id32, id32_d)
            WqT_sb = consts.tile([64, 64], F32)
            nc.sync.dma_start(WqT_sb, WqT_d)
            WkT_sb = consts.tile([64, 64], F32)
            nc.sync.dma_start(WkT_sb, WkT_d)
            WvT_sb = consts.tile([64, 64], F32)
            nc.sync.dma_start(WvT_sb, WvT_d)
            WoT_sb = consts.tile([64, H, DIM], BF16)
            nc.sync.dma_start(WoT_sb, WoT_v)

            # resident transposed routing weights: [128, tile, chunk, tok] bf16
            swT_store = store.tile([128, T, 4, TOK], BF16)
            # slice-token accumulators: st_ps[j][g, jj, 0:64]=st_un(h=4j+jj),
            # col 64 = snorm
            st_ps = [psacc.tile([64, 4, D + 1], F32, name=f"st_ps{j}")
                     for j in range(2)]

            # ================= PASS 1 =================
            xt2 = None
            for t in range(T):
                if t % 2 == 0:
                    xt2 = work.tile([128, KCH, 2 * TOK], BF16, tag="xt2")
                    nc.sync.dma_start(xt2, xT_v[:, :, t * TOK:(t + 2) * TOK])
                xt = xt2[:, :, (t % 2) * TOK:(t % 2 + 1) * TOK]

                # interleave so both matmuls of chunk k reuse the loaded xt[k]
                lg = psmm.tile([128, H, G], F32, tag="lg")
                fxp = psmm.tile([128, H, D], F32, tag="fx")
                for k in range(KCH):
                    nc.tensor.matmul(lg, xt[:, k, :], AT_sb[:, k, :],
                                     start=(k == 0), stop=(k == KCH - 1))
                    nc.tensor.matmul(fxp, xt[:, k, :], WfxT_sb[:, k, :],
                                     start=(k == 0), stop=(k == KCH - 1))

                usw = work.tile([128, H, G], BF16, tag="usw")
                nc.scalar.activation(usw, lg, AF.Exp)
                den = small.tile([128, H], F32, tag="den")
                nc.vector.reduce_sum(den, usw, axis=mybir.AxisListType.X)
                rden = small.tile([128, H], F32, tag="rden")
                nc.vector.reciprocal(rden, den)
                swn = work.tile([128, H, G], BF16, tag="swn")
                nc.gpsimd.tensor_tensor(
                    swn, usw, rden[:, :, None].to_broadcast([128, H, G]), ALU.mult)

                fxs = work.tile([128, H, D + 1], BF16, tag="fxs")
                nc.scalar.copy(fxs[:, 0:4, 0:D], fxp[:, 0:4, :])
                nc.vector.tensor_copy(fxs[:, 4:8, 0:D], fxp[:, 4:8, :])
                nc.gpsimd.memset(fxs[:, :, D], 1.0)

                swn2 = swn.rearrange("p h g -> p (h g)")
                # one accumulation group per PSUM bank (2KB zero region):
                # start only on the first matmul ever touching the bank,
                # stop only on the last.
                for h in range(H):
                    nc.tensor.matmul(st_ps[h // 4][:, h % 4, :],
                                     swn2[:, ts(h, G)], fxs[:, h, :],
                                     start=(t == 0 and h % 4 == 0),
                                     stop=(t == T - 1 and h % 4 == 3))

                if debug and t == 0:
                    nc.sync.dma_start(dbg_swn_d, swn2)
                    nc.sync.dma_start(dbg_usw_d, usw.rearrange("p h g -> p (h g)"))
                swt = psmm.tile([128, 4, TOK], BF16, tag="swt")
                for ci in range(4):
                    nc.tensor.transpose(swt[:, ci, :], swn2[:, ts(ci, 128)], idbf)
                nc.vector.tensor_copy(swT_store[:, t, :, :], swt)

            # ================= STAGE (slice attention, tiny) =================
            stun = stg_pool.tile([64, 2, 4, D + 1], F32)
            nc.vector.tensor_copy(stun[:, 0], st_ps[0])
            nc.vector.tensor_copy(stun[:, 1], st_ps[1])

            cc_in = dram.tile([64, 2 * 4 * (D + 1)], F32)
            cc_out = dram.tile([64, 2 * 4 * (D + 1)], F32)
            nc.sync.dma_start(cc_in, stun.rearrange("p a b c -> p (a b c)"))
            nc.gpsimd.collective_compute(
                "AllReduce", ALU.add,
                replica_groups=[[0, 1], [2, 3], [4, 5], [6, 7]],
                ins=[cc_in.opt()], outs=[cc_out.opt()],
            )
            stg = stg_pool.tile([64, H, D + 1], F32)
            nc.sync.dma_start(stg.rearrange("p h e -> p (h e)"), cc_out)

            if debug:
                nc.sync.dma_start(dbg_stg_d, stg.rearrange("p h e -> p (h e)"))
            snorm_e = stg_pool.tile([64, H], F32)
            nc.vector.tensor_scalar_add(snorm_e, stg[:, :, D], EPS_SLICE)
            rs = stg_pool.tile([64, H], F32)
            nc.vector.reciprocal(rs, snorm_e)
            st_sb = stg_pool.tile([64, H, D], F32)
            nc.vector.tensor_tensor(st_sb, stg[:, :, 0:D],
                                    rs[:, :, None].to_broadcast([64, H, D]),
                                    ALU.mult)
            kv = stg_pool.tile([64, D], F32)
            nc.vector.reduce_sum(kv, st_sb.rearrange("p h d -> p d h"),
                                 axis=mybir.AxisListType.X)

            # transposes of st and kv (f32, 64x64)
            stT = stg_pool.tile([64, H, D], F32)
            for h in range(H):
                tp = psmm.tile([64, 64], F32, tag="swt")
                nc.tensor.transpose(tp, st_sb[:, h, :], id32)
                nc.vector.tensor_copy(stT[:, h, :], tp)
            kvT_p = psmm.tile([64, 64], F32, tag="swt")
            nc.tensor.transpose(kvT_p, kv, id32)
            kvT = stg_pool.tile([64, D], F32)
            nc.vector.tensor_copy(kvT, kvT_p)

            # q = st @ WqT (per head), k/v from kv
            q_ps = psmm.tile([64, H, D], F32, tag="lg")
            for h in range(H):
                nc.tensor.matmul(q_ps[:, h, :], stT[:, h, :], WqT_sb,
                                 start=(h == 0), stop=(h == H - 1))
            k_ps = psmm.tile([64, D], F32, tag="fx")
            nc.tensor.matmul(k_ps, kvT, WkT_sb, start=True, stop=True)
            v_ps = psmm.tile([64, D], F32, tag="fx")
            nc.tensor.matmul(v_ps, kvT, WvT_sb, start=True, stop=True)
            v_sb = stg_pool.tile([64, D], F32)
            nc.vector.tensor_copy(v_sb, v_ps)

            def rnorm(src_ps, nh, tag):
                # 1/sqrt(sum(src^2 over last dim)) with one Newton step
                sq = stg_pool.tile([64, nh, D], F32, name=f"sq_{tag}")
                nc.scalar.activation(sq, src_ps, AF.Square)
                n2 = stg_pool.tile([64, nh], F32, name=f"n2_{tag}")
                nc.vector.reduce_sum(n2, sq, axis=mybir.AxisListType.X)
                r0 = stg_pool.tile([64, nh], F32, name=f"r0_{tag}")
                nc.vector.reciprocal(r0, n2)
                y0 = stg_pool.tile([64, nh], F32, name=f"y0_{tag}")
                nc.scalar.activation(y0, r0, AF.Sqrt)
                t1 = stg_pool.tile([64, nh], F32, name=f"t1_{tag}")
                nc.vector.tensor_mul(t1, y0, y0)
                nc.vector.tensor_mul(t1, t1, n2)
                nc.vector.tensor_scalar(t1, t1, -0.5, 1.5, ALU.mult, ALU.add)
                nc.vector.tensor_mul(t1, t1, y0)
                return t1

            rq = rnorm(q_ps, H, "q")
            rk = rnorm(k_ps[:, None, :], 1, "k")

            qn = stg_pool.tile([64, H, D], F32)
            nc.vector.tensor_tensor(qn, q_ps,
                                    rq[:, :, None].to_broadcast([64, H, D]),
                                    ALU.mult)
            kn = stg_pool.tile([64, D], F32)
            nc.vector.tensor_tensor(kn, k_ps,
                                    rk[:, 0:1].to_broadcast([64, D]), ALU.mult)

            qnT = stg_pool.tile([64, H, D], F32)
            for h in range(H):
                tp = psmm.tile([64, 64], F32, tag="swt")
                nc.tensor.transpose(tp, qn[:, h, :], id32)
                nc.vector.tensor_copy(qnT[:, h, :], tp)
            knT_p = psmm.tile([64, 64], F32, tag="swt")
            nc.tensor.transpose(knT_p, kn, id32)
            knT = stg_pool.tile([64, D], F32)
            nc.vector.tensor_copy(knT, knT_p)

            # attention logits both orientations, exp, denominators
            L_ps = psmm.tile([64, H, G], F32, tag="lg")
            for h in range(H):
                nc.tensor.matmul(L_ps[:, h, :], qnT[:, h, :], knT,
                                 start=(h == 0), stop=(h == H - 1))
            e_sb = stg_pool.tile([64, H, G], F32)
            nc.scalar.activation(e_sb, L_ps, AF.Exp, scale=attn_scale)
            aden = stg_pool.tile([64, H], F32)
            nc.vector.reduce_sum(aden, e_sb, axis=mybir.AxisListType.X)
            ra = stg_pool.tile([64, H], F32)
            nc.vector.reciprocal(ra, aden)

            LT_ps = psmm.tile([64, H, G], F32, tag="fx")
            for h in range(H):
                nc.tensor.matmul(LT_ps[:, h, :], knT, qnT[:, h, :],
                                 start=(h == 0), stop=(h == H - 1))
            eT_sb = stg_pool.tile([64, H, G], F32)
            nc.scalar.activation(eT_sb, LT_ps, AF.Exp, scale=attn_scale)

            av_ps = psmm.tile([64, H, D], F32, tag="lg")
            for h in range(H):
                nc.tensor.matmul(av_ps[:, h, :], eT_sb[:, h, :], v_sb,
                                 start=(h == 0), stop=(h == H - 1))

            os_sb = stg_pool.tile([64, H, D], F32)
            nc.vector.tensor_tensor(os_sb, av_ps,
                                    ra[:, :, None].to_broadcast([64, H, D]),
                                    ALU.mult)
            rst = stg_pool.tile([64, H, D], F32)
            nc.vector.tensor_scalar_mul(rst, st_sb, res_scale)
            nc.vector.tensor_add(os_sb, os_sb, rst)

            osT = stg_pool.tile([64, H, D], BF16)
            for h in range(H):
                tp = psmm.tile([64, 64], F32, tag="swt")
                nc.tensor.transpose(tp, os_sb[:, h, :], id32)
                nc.vector.tensor_copy(osT[:, h, :], tp)

            C_sb = stg_pool.tile([128, 4, DIM], BF16)
            for j in range(4):
                C_ps = psmm.tile([128, DIM], F32, tag="swt")
                for par in range(2):
                    h = 2 * j + par
                    nc.tensor.matmul(C_ps[64 * par:64 * par + 64, :],
                                     osT[:, h, :], WoT_sb[:, h, :],
                                     start=True, stop=True)
                nc.vector.tensor_copy(C_sb[:, j, :], C_ps)

            if debug:
                nc.sync.dma_start(dbg_C_d, C_sb.rearrange("p a b -> p (a b)"))
                nc.sync.dma_start(dbg_ost_d, os_sb.rearrange("p a b -> p (a b)"))
            # ================= PASS 2 =================
            for t in range(T):
                op = psmm.tile([128, DIM], F32, tag="lg")
                for cc in range(4):
                    nc.tensor.matmul(op, swT_store[:, t, cc, :], C_sb[:, cc, :],
                                     start=(cc == 0), stop=(cc == 3))
                ob = work.tile([128, DIM], F32, tag="ob")
                nc.vector.tensor_copy(ob, op)
                nc.sync.dma_start(out_v[t], ob)

    nc.finalize()
    return nc


def kernel(x, Wfx, bfx, Wx, bx, Wslice, bslice, temp, Wq, Wk, Wv,
           res_scale, attn_scale, Wout, bout):
    x = np.asarray(x, dtype=np.float32)
    Wfx = np.asarray(Wfx, np.float32); bfx = np.asarray(bfx, np.float32)
    Wx = np.asarray(Wx, np.float32); bx = np.asarray(bx, np.float32)
    Wslice = np.asarray(Wslice, np.float32); bslice = np.asarray(bslice, np.float32)
    temp = np.asarray(temp, np.float32).reshape(H)
    Wq = np.asarray(Wq, np.float32); Wk = np.asarray(Wk, np.float32)
    Wv = np.asarray(Wv, np.float32)
    res_scale_f = float(np.asarray(res_scale, np.float32))
    attn = np.asarray(attn_scale, np.float32).reshape(H)
    Wout = np.asarray(Wout, np.float32); bout = np.asarray(bout, np.float32)

    assert np.all(np.abs(bfx) == 0) and np.all(np.abs(bx) == 0) \
        and np.all(np.abs(bslice) == 0), "nonzero projection biases unsupported"
    assert np.ptp(attn) == 0, "non-uniform attn_scale unsupported"
    attn_f = float(attn[0])

    # folded logits weight: logits[:, h*G+g] = x @ ((Wslice @ Wx_h)/temp_h).T
    A = np.concatenate(
        [(Wslice @ Wx[h * D:(h + 1) * D, :]) / temp[h] for h in range(H)], axis=0)
    AT = np.ascontiguousarray(A.T).astype(ml_dtypes.bfloat16)          # [256, 512]
    WfxT = np.ascontiguousarray(Wfx.T).astype(ml_dtypes.bfloat16)      # [256, 512]
    WoT = np.ascontiguousarray(Wout.T).astype(ml_dtypes.bfloat16)      # [512, 256]
    WqT = np.ascontiguousarray(Wq.T)
    WkT = np.ascontiguousarray(Wk.T) / H
    WvT = np.ascontiguousarray(Wv.T) / H
    idbf = np.eye(128, dtype=np.float32).astype(ml_dtypes.bfloat16)
    id32 = np.eye(64, dtype=np.float32)

    key = (attn_f, res_scale_f)
    if key not in _CACHE:
        _CACHE[key] = _build(attn_f, res_scale_f)
    nc = _CACHE[key]

    in_maps = []
    for c in range(NCORES):
        b, half = c // 2, c % 2
        xs = x[b, half * NLOC:(half + 1) * NLOC, :]       # [16384, 256]
        xT = np.ascontiguousarray(xs.T.astype(ml_dtypes.bfloat16))
        in_maps.append(dict(xT=xT, AT=AT, WfxT=WfxT, idbf=idbf, id32=id32,
                            WqT=WqT, WkT=WkT, WvT=WvT, WoT=WoT))

    global _LAST_IN_MAPS
    _LAST_IN_MAPS = in_maps
    res = bass_utils.run_bass_kernel_spmd(nc, in_maps, core_ids=list(range(NCORES)))

    out = np.empty((B, N, DIM), np.float32)
    for c in range(NCORES):
        b, half = c // 2, c % 2
        out[b, half * NLOC:(half + 1) * NLOC, :] = res.results[c]["out"]
    if np.any(bout):
        out += bout
    return out

